# revision 19
# baseline (speedup 1.0000x reference)
"""Trainium2 Bass kernel for masked additive (Bahdanau-style) attention.

Computes, for each batch b:
    ph    = h_b @ U                     (T, H)
    e     = tanh(ph + s_b @ W) @ v      (T,)
    e     = where(mask, e, -1e9)
    score = softmax(e)                  (T,)
    ctx   = sum_t score_t * h_b[t]      (D,)

Key observations baked into the kernel:
  * Masked timesteps contribute EXACTLY zero to the output, so the host
    packs only the unmasked timesteps of each batch into a contiguous
    stream.  Batches are SORTED by unmasked count and dealt to 8 cores x
    4 slots so that slot s holds batches of similar count on every core;
    each slot gets its own compiled tile plan (width = per-slot max
    count, ceiled to 128).  For the ~50% random mask this packs 4352 of
    8192 columns per core (53%) vs 4608 for a uniform max-based plan.
    Padding columns carry mask=0 and reuse the masked-softmax path, so
    they contribute exactly 0.  The module is compiled per distinct
    width tuple (cached).
  * proj_s = s @ W is 0.4% of the FLOPs and depends only on (s, W), so
    the host computes it in fp32 as input preprocessing; the device
    reads the per-(batch, H) bias directly.
  * The big matmul (h @ U) is computed transposed: ph^T tiles with H on
    partitions, so the per-batch bias is a per-partition scalar that
    fuses into the tanh activation for free.
  * v-dot split VDP=4/4: chunks 0-3 ride the PE (stationary = v
    broadcast across 128 columns), chunks 4-7 ride the DVE as a
    scalar_tensor_tensor chain whose partition-sum folds into the same
    PSUM group via one all-ones matmul.  PE runs 69 cycles/column; the
    DVE sits at ~69% of the PE's per-tile time (chain+softmax+context),
    so the post-matmul pipeline never backlogs into a long tail.  (The
    Pool engine cannot run TensorScalarPtr -- walrus ISA check -- so it
    carries DMA only.)
  * The softmax + context run flash-style per T-tile (local max/sum +
    fused multiply-accumulate over the resident h^T tile on the DVE),
    so no h tile is ever touched twice.
  * All 2-byte tensors are fp16 (NOT bf16): same PE rate and DMA bytes,
    3 extra mantissa bits.  h^T, U, tanh, chain, ex, scratch fp16; PSUM
    accumulation, et (holds e+512: ulp_f16(512)=0.5 would wreck it) and
    softmax statistics fp32.
  * Startup: the first slot's tile plan leads with a 256-column tile so
    the PE's first matmul group is gated on only ~0.75MB; each DMA ring
    leads with one piece of it, then U chunks stream in exactly the
    order tile 0's mc-sweep consumes them (even chunks on the sync
    ring, odd on scalar).  Steady-state h tiles ride the gpsimd ring;
    batch b+1's DMAs are emitted BEFORE batch b's compute and SBUF
    holds all of a core's packed h (one buffer per full tile), so every
    transfer is in flight long before it is needed.

Sharding: pure data parallelism, 4 batches per core on 8 cores; no
collectives.  Host-side prep shards, packs unmasked timesteps, computes
proj_s, and re-lays-out inputs (transpose of packed h, fp16 casts).
"""

import math

import numpy as np

import concourse.bass as bass
import concourse.tile as tile
from concourse import bacc, mybir
from concourse.bass_utils import run_bass_kernel_spmd

F32 = mybir.dt.float32
F16 = mybir.dt.float16

B, T, D, H = 32, 2048, 1024, 1024
NCORES = 8
BL = B // NCORES          # batches per core (slots)
P = 128                   # partitions
KC = D // P               # 8 contraction chunks
MC = H // P               # 8 output-row chunks
TT = 512                  # max T tile (one PSUM bank)
VDP = 2                   # v-dot chunks on the PE; MC-VDP on the DVE chain
AF = mybir.ActivationFunctionType
ALU = mybir.AluOpType

UCHUNKS = [(128 * i, 128 * (i + 1)) for i in range(MC)]


def _bcast_part(ap, parts=P):
    """Broadcast a 1-partition AP across `parts` partitions (step 0)."""
    return bass.AP(tensor=ap.tensor, offset=ap.offset, ap=[[0, parts]] + list(ap.ap))


def _tile_widths(width, lead256=False, tail128=False):
    """Tile plan for a packed slot width (multiple of 128).

    lead256: start with a 256-col tile so the first PE matmul group is
    gated on a quarter of the usual DMA bytes (used for slot 0).
    tail128: end with [128, 128] and cap the tile before them at <=384
    (used for the LAST slot: each tile's 8 context-accumulates then hide
    under the next tile's mains, so the post-matmul drain is short).
    """
    if lead256 and width >= 640:
        rest = width - 256
        ws = [256] + [TT] * (rest // TT)
        if rest % TT:
            ws.append(rest % TT)
        return ws
    if tail128 and width >= 512:
        rest = width - 256
        ws = [TT] * (rest // TT)
        if rest % TT:
            ws.append(rest % TT)
        return ws + [128, 128]
    ws = [TT] * (width // TT)
    if width % TT:
        ws.append(width % TT)
    return ws


def plan_from_mask(mask):
    """Sort batches by unmasked count, deal to NCORES x BL slots.

    Returns (widths, assign): widths[s] is slot s's packed width;
    assign[c][s] is the global batch index at (core c, slot s).
    """
    cnt = np.asarray(mask).astype(bool).sum(axis=1)
    order = np.argsort(-cnt, kind="stable")
    groups = [order[g * NCORES : (g + 1) * NCORES] for g in range(BL)]
    gw = [
        min(T, max(128, int(math.ceil(int(cnt[g].max()) / 128.0) * 128)))
        for g in groups
    ]
    # slot order within a core: lead and trail with the two largest
    # groups so the last batch ends on a short partial tile.
    slot_order = [0] + list(range(2, BL)) + [1] if BL >= 2 else [0]
    widths = tuple(gw[i] for i in slot_order)
    assign = [[int(groups[i][c]) for i in slot_order] for c in range(NCORES)]
    return widths, assign


def _plans(widths):
    return [
        _tile_widths(w, lead256=(bl == 0), tail128=(bl == len(widths) - 1))
        for bl, w in enumerate(widths)
    ]


def build_module(widths):
    widths_of = _plans(widths)
    offs_of = [[sum(ws[:i]) for i in range(len(ws))] for ws in widths_of]
    nt_of = [len(ws) for ws in widths_of]

    nc = bacc.Bacc(
        "TRN2",
        target_bir_lowering=False,
        debug=False,
        enable_asserts=False,
        num_devices=NCORES,
    )

    # hT arrives pre-tiled: one contiguous (P, KC, w) tensor per T-tile,
    # and U pre-chunked into contiguous (P, KC, cols) column groups.  Both
    # give per-partition-contiguous 2-8KB DMA runs.
    hTt = [
        [
            nc.dram_tensor(
                f"hT{bl}_{tt}", [P, KC, w], F16, kind="ExternalInput"
            ).ap()
            for tt, w in enumerate(widths_of[bl])
        ]
        for bl in range(BL)
    ]
    Uc = [
        nc.dram_tensor(f"U{i}", [P, KC, hi - lo], F16, kind="ExternalInput").ap()
        for i, (lo, hi) in enumerate(UCHUNKS)
    ]
    # proj/v arrive pre-laid-out partition-major.
    proj = nc.dram_tensor("proj", [P, MC * BL], F32, kind="ExternalInput").ap()
    maskf = [
        nc.dram_tensor(f"maskf{bl}", [widths[bl]], F16, kind="ExternalInput").ap()
        for bl in range(BL)
    ]
    v = nc.dram_tensor("v", [P, MC], F32, kind="ExternalInput").ap()
    # out[b, p*KC + dc] = ctx[dc*128 + p]: per-partition-contiguous 32B
    # runs; the host unscrambles.
    out = nc.dram_tensor("out", [BL, P * KC], F32, kind="ExternalOutput").ap()

    with tile.TileContext(nc) as tc:
        with (
            tc.tile_pool(name="singles", bufs=1) as singles,
            tc.tile_pool(name="ht", bufs=7) as ht_pool,
            tc.tile_pool(name="htp", bufs=2) as htp_pool,
            tc.tile_pool(name="mask", bufs=2) as mask_pool,
            tc.tile_pool(name="tanh", bufs=10) as tanh_pool,
            tc.tile_pool(name="vd", bufs=4) as vd_pool,
            tc.tile_pool(name="p2", bufs=3) as p2_pool,
            tc.tile_pool(name="scr", bufs=2) as scr_pool,
            tc.tile_pool(name="small", bufs=4) as small_pool,
            tc.tile_pool(name="ctx", bufs=2) as ctx_pool,
            tc.tile_pool(name="ps", bufs=7, space="PSUM") as ps_pool,
            tc.tile_pool(name="eps", bufs=1, space="PSUM") as e_pool,
        ):
            # ---- persistent operands -------------------------------------
            # Three DMA rings (gpsimd / sync / scalar), each drains in
            # issue order.  Tile 0 (256 cols) is split three ways so every
            # ring leads with a piece of the first matmul group's data;
            # U0 follows immediately on sync, then U chunks alternate
            # sync/scalar in mc order so chunk mc lands just before the
            # tile-0 sweep needs it.
            def ht_tile(b, tt):
                w = widths_of[b][tt]
                if w == TT:
                    return ht_pool.tile([P, KC, w], F16, tag="ht", name=f"ht_b{b}t{tt}")
                return htp_pool.tile(
                    [P, KC, w], F16, tag=f"htp{w}", name=f"ht_b{b}t{tt}"
                )

            ht0_tiles = [ht_tile(0, tt) for tt in range(nt_of[0])]
            u_tiles = [
                singles.tile([P, KC, hi - lo], F16, name=f"u_sb{i}")
                for i, (lo, hi) in enumerate(UCHUNKS)
            ]
            # Critical startup bytes (t0 0.5MB + U 2MB + t1 1MB + proj) are
            # spread across the three rings; tile 0's mc sweep order
            # (MC0_ORDER) matches the U-chunk arrival order so the PE never
            # stalls.  A dma_start BLOCKS its queue until the transfer
            # completes, so the scalar (Act) queue gets only the minimal
            # share that drains before the first tanh is needed -- all
            # later traffic rides sync (SP, no compute) and gpsimd (Pool).
            nc.gpsimd.dma_start(
                out=ht0_tiles[0][:, 0:3, :], in_=hTt[0][0][:, 0:3, :]
            )
            nc.sync.dma_start(
                out=ht0_tiles[0][:, 3:6, :], in_=hTt[0][0][:, 3:6, :]
            )
            nc.scalar.dma_start(
                out=ht0_tiles[0][:, 6:8, :], in_=hTt[0][0][:, 6:8, :]
            )
            # proj + v gate b0t0's tanh -> chain -> PSUM frees.
            proj_sb = singles.tile([P, MC, BL], F32)
            nc.scalar.dma_start(
                out=proj_sb, in_=proj.rearrange("p (mc b) -> p mc b", mc=MC)
            )
            v_col = singles.tile([P, MC], F32)
            nc.scalar.dma_start(out=v_col, in_=v)
            # Chain-side chunks (mc >= VDP) lead so tanh->chain starts
            # early and frees PSUM banks; PE-side chunks (mc 0,1) arrive
            # last, matching the end of tile 0's sweep.
            nc.gpsimd.dma_start(out=u_tiles[7], in_=Uc[7])
            nc.sync.dma_start(out=u_tiles[5], in_=Uc[5])
            nc.scalar.dma_start(out=u_tiles[2], in_=Uc[2])
            nc.gpsimd.dma_start(out=u_tiles[0], in_=Uc[0])
            nc.sync.dma_start(out=u_tiles[6], in_=Uc[6])
            nc.scalar.dma_start(out=u_tiles[3], in_=Uc[3])
            nc.gpsimd.dma_start(out=u_tiles[1], in_=Uc[1])
            nc.scalar.dma_start(out=u_tiles[4], in_=Uc[4])
            # b0's later tiles behind U on gpsimd/sync only.
            if nt_of[0] > 1:
                nc.gpsimd.dma_start(
                    out=ht0_tiles[1][:, 0:4, :], in_=hTt[0][1][:, 0:4, :]
                )
                nc.sync.dma_start(
                    out=ht0_tiles[1][:, 4:6, :], in_=hTt[0][1][:, 4:6, :]
                )
                nc.sync.dma_start(
                    out=ht0_tiles[1][:, 6:8, :], in_=hTt[0][1][:, 6:8, :]
                )
            for tt in range(2, nt_of[0]):
                nc.gpsimd.dma_start(out=ht0_tiles[tt], in_=hTt[0][tt])

            # v-dot PE-side stationaries: v broadcast across 128 columns.
            v_bc = singles.tile([P, VDP, P], F16)
            for mc in range(VDP):
                nc.vector.memset(v_bc[:, mc, :], 0.0)
                nc.vector.tensor_scalar_add(
                    out=v_bc[:, mc, :],
                    in0=v_bc[:, mc, :],
                    scalar1=v_col[:, mc : mc + 1],
                )
            ones_bc = singles.tile([P, P], F16)
            nc.vector.memset(ones_bc, 1.0)

            # ---- emission helpers -----------------------------------------
            def emit_batch_dmas(b, pre_tiles=None):
                ht_tiles = []
                for tt in range(nt_of[b]):
                    if pre_tiles is not None:
                        ht_tiles.append(pre_tiles[tt])
                        continue
                    htt = ht_tile(b, tt)
                    # h for the middle batches rides gpsimd; the last
                    # batch rides sync (drained of masks by then).  The
                    # scalar queue carries NO steady-state DMA at all.
                    eng = nc.sync if b == BL - 1 else nc.gpsimd
                    eng.dma_start(out=htt, in_=hTt[b][tt])
                    ht_tiles.append(htt)
                mb_sb = mask_pool.tile(
                    [P, widths[b]], F16, tag=f"m{widths[b]}", name=f"mb{b}"
                )
                nc.sync.dma_start(out=mb_sb, in_=_bcast_part(maskf[b]))
                return ht_tiles, mb_sb

            # tile-0 mc sweep order matches U-chunk ring arrival order.
            MC0_ORDER = [7, 5, 2, 0, 6, 3, 1, 4]

            def emit_mains(b, tt, ht_tiles, mc_order=None):
                w = widths_of[b][tt]
                pps = [None] * MC
                for mc in mc_order or range(MC):
                    pp = ps_pool.tile(
                        [P, TT], F32, tag="ps", name=f"pp{b}_{tt}_{mc}"
                    )
                    for kc in range(KC):
                        nc.tensor.matmul(
                            pp[:, :w],
                            lhsT=u_tiles[mc][:, kc, :],
                            rhs=ht_tiles[tt][:, kc, :],
                            start=(kc == 0),
                            stop=(kc == KC - 1),
                        )
                    pps[mc] = pp
                return pps

            def emit_tile_soft(b, tt, pps, ht_tiles, mb_sb, st):
                # tanh + v-dot, then the online-softmax tile pass:
                #   et  = (e + 512) * m   (masked/pad -> 0; 512 > max|e| and
                #         exp(-512-max) underflows to exactly 0 in fp32,
                #         while ulp_f32(512)=6.1e-5 keeps e's precision)
                #   nmax_i = -max(et); ex = exp(et - max_i); z_i = sum(ex)
                #   part[:, dc, i] = sum_t ex_t * hT[p, dc, t]
                w = widths_of[b][tt]
                nmax, zs, _, _ = st
                e_ps = e_pool.tile([P, TT], F32, tag="e", name=f"e{b}_{tt}")
                ths = [None] * MC

                def emit_tanh(mc):
                    th = tanh_pool.tile(
                        [P, TT], F16, tag="th", name=f"th{b}_{tt}_{mc}"
                    )
                    nc.scalar.activation(
                        out=th[:, :w],
                        in_=pps[mc][:, :w],
                        func=AF.Tanh,
                        bias=proj_sb[:, mc, b : b + 1],
                        scale=1.0,
                    )
                    ths[mc] = th

                # DVE-side chunks FIRST: their tanh->chain path gates the
                # all-ones matmul, which heads the e_ps PSUM group
                # (start=True); the PE-side v-dots close the group.
                for mc in range(VDP, MC):
                    emit_tanh(mc)
                ea = vd_pool.tile([P, TT], F16, tag="ea", name=f"ea{b}_{tt}_a")
                nc.vector.tensor_scalar_mul(
                    out=ea[:, :w],
                    in0=ths[VDP][:, :w],
                    scalar1=v_col[:, VDP : VDP + 1],
                )
                for k in range(VDP + 1, MC):
                    ea2 = vd_pool.tile(
                        [P, TT], F16, tag="ea", name=f"ea{b}_{tt}_{k}"
                    )
                    nc.vector.scalar_tensor_tensor(
                        out=ea2[:, :w],
                        in0=ths[k][:, :w],
                        scalar=v_col[:, k : k + 1],
                        in1=ea[:, :w],
                        op0=ALU.mult,
                        op1=ALU.add,
                    )
                    ea = ea2
                nc.tensor.matmul(
                    e_ps[:, :w],
                    lhsT=ones_bc,
                    rhs=ea[:, :w],
                    start=True,
                    stop=False,
                )
                for mc in range(VDP):
                    emit_tanh(mc)
                    nc.tensor.matmul(
                        e_ps[:, :w],
                        lhsT=v_bc[:, mc, :],
                        rhs=ths[mc][:, :w],
                        start=False,
                        stop=(mc == VDP - 1),
                    )
                # et is F32: it holds e+512, and ulp_f16(512)=0.5 would
                # quantize the energy to +-0.25.
                et = p2_pool.tile([P, TT], F32, tag="et", name=f"et{b}_{tt}")
                nc.vector.scalar_tensor_tensor(
                    out=et[:, :w],
                    in0=e_ps[:, :w],
                    scalar=512.0,
                    in1=mb_sb[:, offs_of[b][tt] : offs_of[b][tt] + w],
                    op0=ALU.add,
                    op1=ALU.mult,
                )
                nc.vector.tensor_reduce(
                    out=nmax[:, tt : tt + 1],
                    in_=et[:, :w],
                    axis=mybir.AxisListType.X,
                    op=ALU.max,
                    negate=True,
                )
                ex = p2_pool.tile([P, TT], F16, tag="ex", name=f"ex{b}_{tt}")
                nc.scalar.activation(
                    out=ex[:, :w],
                    in_=et[:, :w],
                    func=AF.Exp,
                    bias=nmax[:, tt : tt + 1],
                    scale=1.0,
                    accum_out=zs[:, tt : tt + 1],
                )
                return ex

            def emit_tile_ctx(b, tt, ht_tiles, ex, st):
                # deferred one tile behind the softmax head so the next
                # tile's et/nmax (critical path to its exp) queue ahead of
                # these 8 accumulates on the DVE.
                w = widths_of[b][tt]
                _, _, part, scr = st
                for dc in range(KC):
                    nc.vector.scalar_tensor_tensor(
                        out=scr[:, :w],
                        in0=ht_tiles[tt][:, dc, :],
                        scalar=1.0,
                        in1=ex[:, :w],
                        op0=ALU.mult,
                        op1=ALU.mult,
                        accum_out=part[:, dc, tt : tt + 1],
                    )

            def emit_batch_tail(b, st):
                # combine tiles: f_i = exp(max_i - M) with global max M,
                # ctx = sum_i part_i f_i / sum_i z_i f_i  (all tiny tiles)
                nt = nt_of[b]
                nmax, zs, part, _ = st
                negM = small_pool.tile([P, 1], F32, tag="negM", name=f"nM{b}")
                nc.vector.tensor_reduce(
                    out=negM, in_=nmax, axis=mybir.AxisListType.X, op=ALU.min
                )
                f = small_pool.tile([P, nt], F32, tag=f"f{nt}", name=f"f{b}")
                nc.scalar.activation(
                    out=f, in_=nmax, func=AF.Exp, bias=negM, scale=-1.0
                )
                fz = small_pool.tile([P, nt], F32, tag=f"fz{nt}", name=f"fz{b}")
                zf = small_pool.tile([P, 1], F32, tag="zf", name=f"zf{b}")
                nc.vector.scalar_tensor_tensor(
                    out=fz,
                    in0=zs,
                    scalar=1.0,
                    in1=f,
                    op0=ALU.mult,
                    op1=ALU.mult,
                    accum_out=zf,
                )
                sinv = small_pool.tile([P, 1], F32, tag="sinv", name=f"si{b}")
                nc.vector.reciprocal(sinv, zf)
                for tt in range(nt):
                    nc.vector.tensor_scalar_mul(
                        out=part[:, :, tt : tt + 1],
                        in0=part[:, :, tt : tt + 1],
                        scalar1=f[:, tt : tt + 1],
                    )
                ctx = ctx_pool.tile([P, KC], F32, tag="ctx", name=f"cx{b}")
                nc.vector.tensor_reduce(
                    out=ctx, in_=part, axis=mybir.AxisListType.X, op=ALU.add
                )
                nc.vector.tensor_scalar_mul(out=ctx, in0=ctx, scalar1=sinv)
                nc.sync.dma_start(
                    out=out[b].rearrange("(p dc) -> p dc", p=P), in_=ctx
                )

            def batch_state(b):
                nt = nt_of[b]
                nmax = small_pool.tile([P, nt], F32, tag=f"nmax{nt}", name=f"nm{b}")
                zs = small_pool.tile([P, nt], F32, tag=f"zs{nt}", name=f"zs{b}")
                part = ctx_pool.tile(
                    [P, KC, nt], F32, tag=f"part{nt}", name=f"pt{b}"
                )
                scr = scr_pool.tile([P, TT], F16, tag="scr", name=f"sc{b}")
                return nmax, zs, part, scr

            # ---- pipeline -------------------------------------------------
            # batch b+1's DMAs are emitted BEFORE batch b's compute so the
            # transfers are in flight long before the PE reaches them.  Each
            # tile's ctx accumulates are emitted AFTER the next tile's
            # softmax head (deferred-ctx: keeps et/nmax ahead on the DVE).
            dmas = {0: emit_batch_dmas(0, pre_tiles=ht0_tiles)}
            pend = None  # (b, tt, ht_tiles, ex, st) awaiting ctx emission
            for b in range(BL):
                if b + 1 < BL:
                    dmas[b + 1] = emit_batch_dmas(b + 1)
                ht_tiles, mb_sb = dmas.pop(b)
                st = batch_state(b)
                for tt in range(nt_of[b]):
                    mc_order = MC0_ORDER if b == 0 and tt == 0 else None
                    pps = emit_mains(b, tt, ht_tiles, mc_order)
                    ex = emit_tile_soft(b, tt, pps, ht_tiles, mb_sb, st)
                    if pend is not None:
                        emit_tile_ctx(*pend)
                    pend = (b, tt, ht_tiles, ex, st)
                emit_tile_ctx(*pend)
                pend = None
                emit_batch_tail(b, st)

    nc.compile()
    return nc


_NC_CACHE = {}


def _get_module(widths):
    if widths not in _NC_CACHE:
        _NC_CACHE[widths] = build_module(widths)
    return _NC_CACHE[widths]


def core_in_map(s, h, mask, W, U, v, c, widths, assign):
    """Shard + pack unmasked timesteps + lay out the inputs for core c.

    hT is delivered pre-tiled: per T-tile contiguous (P, KC, w) tensors
    (partition-contiguous rows -> large DMA packets); U likewise as
    contiguous (P, KC, cols) column chunks.
    """
    plans = _plans(widths)
    im = {}
    proj_rows = np.empty((BL, H), np.float32)
    for bl in range(BL):
        gb = assign[c][bl]
        wd = widths[bl]
        h_b = np.asarray(h, np.float32)[gb]
        m_b = np.asarray(mask)[gb] != 0
        idx = np.nonzero(m_b)[0]
        tb = min(len(idx), wd)
        hT_p = np.zeros((D, wd), dtype=np.float16)
        mf_p = np.zeros((wd,), dtype=np.float16)
        if tb:
            hT_p[:, :tb] = h_b[idx[:tb], :].T.astype(np.float16)
            mf_p[:tb] = 1.0
        # (kc p) rows -> (P, KC, w) tiles
        hr = hT_p.reshape(KC, P, wd)
        ws = plans[bl]
        offs = [sum(ws[:i]) for i in range(len(ws))]
        for tt, w in enumerate(ws):
            im[f"hT{bl}_{tt}"] = np.ascontiguousarray(
                hr[:, :, offs[tt] : offs[tt] + w].transpose(1, 0, 2)
            )
        im[f"maskf{bl}"] = mf_p
        proj_rows[bl] = np.asarray(s, np.float32)[0, gb] @ np.asarray(
            W, np.float32
        )
    Ur = (
        np.asarray(U, np.float32)
        .astype(np.float16)
        .reshape(KC, P, H)
        .transpose(1, 0, 2)
    )
    for i, (lo, hi) in enumerate(UCHUNKS):
        im[f"U{i}"] = np.ascontiguousarray(Ur[:, :, lo:hi])
    # partition-major: proj_l[p, mc*BL + b] = proj[b, mc*128 + p]
    im["proj"] = np.ascontiguousarray(
        proj_rows.T.reshape(MC, P, BL).transpose(1, 0, 2).reshape(P, MC * BL)
    )
    # v_l[p, mc] = v[mc*128 + p]
    im["v"] = np.ascontiguousarray(
        np.asarray(v, np.float32).reshape(MC, P).T
    )
    return im


def unscramble_out(arr):
    """(BL, P*KC) device layout [p, dc] -> (BL, D) with d = dc*128 + p."""
    arr = np.asarray(arr)
    return np.ascontiguousarray(
        arr.reshape(-1, P, KC).transpose(0, 2, 1).reshape(-1, D)
    )


def assemble_out(results, mask, assign):
    """Scatter per-core slot outputs back to global batch order."""
    outp = np.zeros((B, D), np.float32)
    for c in range(NCORES):
        ob = unscramble_out(results[c]["out"])
        for bl in range(BL):
            outp[assign[c][bl]] = ob[bl]
    # fully-masked batches: reference yields exactly 0 (softmax uniform
    # over zeroed h); the device path divides by z=0 there, so overwrite.
    tb = np.asarray(mask).astype(bool).sum(axis=1)
    outp[tb == 0] = 0.0
    return outp


def kernel(s, h, mask, W, U, v):
    widths, assign = plan_from_mask(mask)
    in_maps = [
        core_in_map(s, h, mask, W, U, v, c, widths, assign)
        for c in range(NCORES)
    ]
    nc = _get_module(widths)
    res = run_bass_kernel_spmd(nc, in_maps, list(range(NCORES)))
    return assemble_out(res.results, mask, assign)


# revision 21
# speedup vs baseline: 1.0357x; 1.0357x over previous
"""Trainium2 Bass kernel for masked additive (Bahdanau-style) attention.

Computes, for each batch b:
    ph    = h_b @ U                     (T, H)
    e     = tanh(ph + s_b @ W) @ v      (T,)
    e     = where(mask, e, -1e9)
    score = softmax(e)                  (T,)
    ctx   = sum_t score_t * h_b[t]      (D,)

Key observations baked into the kernel:
  * Masked timesteps contribute EXACTLY zero to the output, so the host
    packs only the unmasked timesteps of each batch into a contiguous
    stream.  Batches are SORTED by unmasked count and dealt to 8 cores x
    4 slots so that slot s holds batches of similar count on every core;
    each slot gets its own compiled tile plan (width = per-slot max
    count, ceiled to 128).  For the ~50% random mask this packs 4352 of
    8192 columns per core (53%) vs 4608 for a uniform max-based plan.
    Padding columns carry mask=0 and reuse the masked-softmax path, so
    they contribute exactly 0.  The module is compiled per distinct
    width tuple (cached).
  * proj_s = s @ W is 0.4% of the FLOPs and depends only on (s, W), so
    the host computes it in fp32 as input preprocessing; the device
    reads the per-(batch, H) bias directly.
  * The big matmul (h @ U) is computed transposed: ph^T tiles with H on
    partitions, so the per-batch bias is a per-partition scalar that
    fuses into the tanh activation for free.
  * v-dot split VDP=4/4: chunks 0-3 ride the PE (stationary = v
    broadcast across 128 columns), chunks 4-7 ride the DVE as a
    scalar_tensor_tensor chain whose partition-sum folds into the same
    PSUM group via one all-ones matmul.  PE runs 69 cycles/column; the
    DVE sits at ~69% of the PE's per-tile time (chain+softmax+context),
    so the post-matmul pipeline never backlogs into a long tail.  (The
    Pool engine cannot run TensorScalarPtr -- walrus ISA check -- so it
    carries DMA only.)
  * The softmax + context run flash-style per T-tile (local max/sum +
    fused multiply-accumulate over the resident h^T tile on the DVE),
    so no h tile is ever touched twice.
  * All 2-byte tensors are fp16 (NOT bf16): same PE rate and DMA bytes,
    3 extra mantissa bits.  h^T, U, tanh, chain, ex, scratch fp16; PSUM
    accumulation, et (holds e+512: ulp_f16(512)=0.5 would wreck it) and
    softmax statistics fp32.
  * Startup: the first slot's tile plan leads with a 256-column tile so
    the PE's first matmul group is gated on only ~0.75MB; each DMA ring
    leads with one piece of it, then U chunks stream in exactly the
    order tile 0's mc-sweep consumes them (even chunks on the sync
    ring, odd on scalar).  Steady-state h tiles ride the gpsimd ring;
    batch b+1's DMAs are emitted BEFORE batch b's compute and SBUF
    holds all of a core's packed h (one buffer per full tile), so every
    transfer is in flight long before it is needed.

Sharding: pure data parallelism, 4 batches per core on 8 cores; no
collectives.  Host-side prep shards, packs unmasked timesteps, computes
proj_s, and re-lays-out inputs (transpose of packed h, fp16 casts).
"""

import math

import numpy as np

import concourse.bass as bass
import concourse.tile as tile
from concourse import bacc, mybir
from concourse.bass_utils import run_bass_kernel_spmd

F32 = mybir.dt.float32
F16 = mybir.dt.float16

B, T, D, H = 32, 2048, 1024, 1024
NCORES = 8
BL = B // NCORES          # batches per core (slots)
P = 128                   # partitions
KC = D // P               # 8 contraction chunks
MC = H // P               # 8 output-row chunks
TT = 512                  # max T tile (one PSUM bank)
VDP = 2                   # v-dot chunks on the PE; MC-VDP on the DVE chain
AF = mybir.ActivationFunctionType
ALU = mybir.AluOpType

UCHUNKS = [(128 * i, 128 * (i + 1)) for i in range(MC)]


def _bcast_part(ap, parts=P):
    """Broadcast a 1-partition AP across `parts` partitions (step 0)."""
    return bass.AP(tensor=ap.tensor, offset=ap.offset, ap=[[0, parts]] + list(ap.ap))


def _fixup(ws):
    """Keep every tile >=128 wide (a <128 tile is ldweights-bound and
    costs like a 128 one): borrow columns from the previous tile."""
    if len(ws) >= 2 and ws[-1] < 128:
        ws[-2] -= 128 - ws[-1]
        ws[-1] = 128
    return ws


def _tile_widths(width, lead256=False, tail128=False):
    """Tile plan for a packed slot width (multiple of 32).

    lead256: start with a 256-col tile so the first PE matmul group is
    gated on a quarter of the usual DMA bytes (used for slot 0).
    tail128: end with [128, 128] and cap the tile before them at <=384
    (used for the LAST slot: each tile's 8 context-accumulates then hide
    under the next tile's mains, so the post-matmul drain is short).
    """
    if lead256 and width >= 640:
        rest = width - 256
        ws = [256] + [TT] * (rest // TT)
        if rest % TT:
            ws.append(rest % TT)
        return _fixup(ws)
    if tail128 and width >= 512:
        rest = width - 256
        ws = [TT] * (rest // TT)
        if rest % TT:
            ws.append(rest % TT)
        return _fixup(ws) + [128, 128]
    ws = [TT] * (width // TT)
    if width % TT:
        ws.append(width % TT)
    return _fixup(ws)


def plan_from_mask(mask):
    """Sort batches by unmasked count, deal to NCORES x BL slots.

    Returns (widths, assign): widths[s] is slot s's packed width;
    assign[c][s] is the global batch index at (core c, slot s).
    """
    cnt = np.asarray(mask).astype(bool).sum(axis=1)
    order = np.argsort(-cnt, kind="stable")
    groups = [order[g * NCORES : (g + 1) * NCORES] for g in range(BL)]
    gw = [
        min(T, max(128, int(math.ceil(int(cnt[g].max()) / 32.0) * 32)))
        for g in groups
    ]
    # slot order within a core: lead and trail with the two largest
    # groups so the last batch ends on a short partial tile.
    slot_order = [0] + list(range(2, BL)) + [1] if BL >= 2 else [0]
    widths = tuple(gw[i] for i in slot_order)
    assign = [[int(groups[i][c]) for i in slot_order] for c in range(NCORES)]
    return widths, assign


def _plans(widths):
    return [
        _tile_widths(w, lead256=(bl == 0), tail128=(bl == len(widths) - 1))
        for bl, w in enumerate(widths)
    ]


def build_module(widths):
    widths_of = _plans(widths)
    offs_of = [[sum(ws[:i]) for i in range(len(ws))] for ws in widths_of]
    nt_of = [len(ws) for ws in widths_of]

    nc = bacc.Bacc(
        "TRN2",
        target_bir_lowering=False,
        debug=False,
        enable_asserts=False,
        num_devices=NCORES,
    )

    # hT arrives pre-tiled: one contiguous (P, KC, w) tensor per T-tile,
    # and U pre-chunked into contiguous (P, KC, cols) column groups.  Both
    # give per-partition-contiguous 2-8KB DMA runs.
    hTt = [
        [
            nc.dram_tensor(
                f"hT{bl}_{tt}", [P, KC, w], F16, kind="ExternalInput"
            ).ap()
            for tt, w in enumerate(widths_of[bl])
        ]
        for bl in range(BL)
    ]
    Uc = [
        nc.dram_tensor(f"U{i}", [P, KC, hi - lo], F16, kind="ExternalInput").ap()
        for i, (lo, hi) in enumerate(UCHUNKS)
    ]
    # proj/v arrive pre-laid-out partition-major.
    proj = nc.dram_tensor("proj", [P, MC * BL], F32, kind="ExternalInput").ap()
    maskf = [
        nc.dram_tensor(f"maskf{bl}", [widths[bl]], F16, kind="ExternalInput").ap()
        for bl in range(BL)
    ]
    v = nc.dram_tensor("v", [P, MC], F32, kind="ExternalInput").ap()
    # out[b, p*KC + dc] = ctx[dc*128 + p]: per-partition-contiguous 32B
    # runs; the host unscrambles.
    out = nc.dram_tensor("out", [BL, P * KC], F32, kind="ExternalOutput").ap()

    with tile.TileContext(nc) as tc:
        with (
            tc.tile_pool(name="singles", bufs=1) as singles,
            tc.tile_pool(name="ht", bufs=7) as ht_pool,
            tc.tile_pool(name="htp", bufs=2) as htp_pool,
            tc.tile_pool(name="mask", bufs=2) as mask_pool,
            tc.tile_pool(name="tanh", bufs=10) as tanh_pool,
            tc.tile_pool(name="vd", bufs=4) as vd_pool,
            tc.tile_pool(name="p2", bufs=3) as p2_pool,
            tc.tile_pool(name="scr", bufs=2) as scr_pool,
            tc.tile_pool(name="small", bufs=4) as small_pool,
            tc.tile_pool(name="ctx", bufs=2) as ctx_pool,
            tc.tile_pool(name="ps", bufs=7, space="PSUM") as ps_pool,
            tc.tile_pool(name="eps", bufs=1, space="PSUM") as e_pool,
        ):
            # ---- persistent operands -------------------------------------
            # Three DMA rings (gpsimd / sync / scalar), each drains in
            # issue order.  Tile 0 (256 cols) is split three ways so every
            # ring leads with a piece of the first matmul group's data;
            # U0 follows immediately on sync, then U chunks alternate
            # sync/scalar in mc order so chunk mc lands just before the
            # tile-0 sweep needs it.
            def ht_tile(b, tt):
                w = widths_of[b][tt]
                if w == TT:
                    return ht_pool.tile([P, KC, w], F16, tag="ht", name=f"ht_b{b}t{tt}")
                return htp_pool.tile(
                    [P, KC, w], F16, tag=f"htp{w}", name=f"ht_b{b}t{tt}"
                )

            ht0_tiles = [ht_tile(0, tt) for tt in range(nt_of[0])]
            u_tiles = [
                singles.tile([P, KC, hi - lo], F16, name=f"u_sb{i}")
                for i, (lo, hi) in enumerate(UCHUNKS)
            ]
            # Critical startup bytes (t0 0.5MB + U 2MB + t1 1MB + proj) are
            # spread across the three rings; tile 0's mc sweep order
            # (MC0_ORDER) matches the U-chunk arrival order so the PE never
            # stalls.  A dma_start BLOCKS its queue until the transfer
            # completes, so the scalar (Act) queue gets only the minimal
            # share that drains before the first tanh is needed -- all
            # later traffic rides sync (SP, no compute) and gpsimd (Pool).
            nc.gpsimd.dma_start(
                out=ht0_tiles[0][:, 0:3, :], in_=hTt[0][0][:, 0:3, :]
            )
            nc.sync.dma_start(
                out=ht0_tiles[0][:, 3:6, :], in_=hTt[0][0][:, 3:6, :]
            )
            nc.scalar.dma_start(
                out=ht0_tiles[0][:, 6:8, :], in_=hTt[0][0][:, 6:8, :]
            )
            # proj + v gate b0t0's tanh -> chain -> PSUM frees.
            proj_sb = singles.tile([P, MC, BL], F32)
            nc.scalar.dma_start(
                out=proj_sb, in_=proj.rearrange("p (mc b) -> p mc b", mc=MC)
            )
            v_col = singles.tile([P, MC], F32)
            nc.scalar.dma_start(out=v_col, in_=v)
            # Chain-side chunks (mc >= VDP) lead so tanh->chain starts
            # early and frees PSUM banks; PE-side chunks (mc 0,1) arrive
            # last, matching the end of tile 0's sweep.
            nc.gpsimd.dma_start(out=u_tiles[7], in_=Uc[7])
            nc.sync.dma_start(out=u_tiles[5], in_=Uc[5])
            nc.scalar.dma_start(out=u_tiles[2], in_=Uc[2])
            nc.gpsimd.dma_start(out=u_tiles[0], in_=Uc[0])
            nc.sync.dma_start(out=u_tiles[6], in_=Uc[6])
            nc.scalar.dma_start(out=u_tiles[3], in_=Uc[3])
            nc.gpsimd.dma_start(out=u_tiles[1], in_=Uc[1])
            nc.scalar.dma_start(out=u_tiles[4], in_=Uc[4])
            # b0's later tiles behind U on gpsimd/sync only.
            if nt_of[0] > 1:
                nc.gpsimd.dma_start(
                    out=ht0_tiles[1][:, 0:4, :], in_=hTt[0][1][:, 0:4, :]
                )
                nc.sync.dma_start(
                    out=ht0_tiles[1][:, 4:6, :], in_=hTt[0][1][:, 4:6, :]
                )
                nc.sync.dma_start(
                    out=ht0_tiles[1][:, 6:8, :], in_=hTt[0][1][:, 6:8, :]
                )
            for tt in range(2, nt_of[0]):
                nc.gpsimd.dma_start(out=ht0_tiles[tt], in_=hTt[0][tt])

            # v-dot PE-side stationaries: v broadcast across 128 columns.
            v_bc = singles.tile([P, VDP, P], F16)
            for mc in range(VDP):
                nc.vector.memset(v_bc[:, mc, :], 0.0)
                nc.vector.tensor_scalar_add(
                    out=v_bc[:, mc, :],
                    in0=v_bc[:, mc, :],
                    scalar1=v_col[:, mc : mc + 1],
                )
            ones_bc = singles.tile([P, P], F16)
            nc.vector.memset(ones_bc, 1.0)

            # ---- emission helpers -----------------------------------------
            def emit_batch_dmas(b, pre_tiles=None):
                ht_tiles = []
                for tt in range(nt_of[b]):
                    if pre_tiles is not None:
                        ht_tiles.append(pre_tiles[tt])
                        continue
                    htt = ht_tile(b, tt)
                    # h for the middle batches rides gpsimd; the last
                    # batch rides sync (drained of masks by then).  The
                    # scalar queue carries NO steady-state DMA at all.
                    eng = nc.sync if b == BL - 1 else nc.gpsimd
                    eng.dma_start(out=htt, in_=hTt[b][tt])
                    ht_tiles.append(htt)
                mb_sb = mask_pool.tile(
                    [P, widths[b]], F16, tag=f"m{widths[b]}", name=f"mb{b}"
                )
                nc.sync.dma_start(out=mb_sb, in_=_bcast_part(maskf[b]))
                return ht_tiles, mb_sb

            # tile-0 mc sweep order matches U-chunk ring arrival order.
            MC0_ORDER = [7, 5, 2, 0, 6, 3, 1, 4]

            def emit_mains(b, tt, ht_tiles, mc_order=None):
                w = widths_of[b][tt]
                pps = [None] * MC
                for mc in mc_order or range(MC):
                    pp = ps_pool.tile(
                        [P, TT], F32, tag="ps", name=f"pp{b}_{tt}_{mc}"
                    )
                    for kc in range(KC):
                        nc.tensor.matmul(
                            pp[:, :w],
                            lhsT=u_tiles[mc][:, kc, :],
                            rhs=ht_tiles[tt][:, kc, :],
                            start=(kc == 0),
                            stop=(kc == KC - 1),
                        )
                    pps[mc] = pp
                return pps

            def emit_tile_soft(b, tt, pps, ht_tiles, mb_sb, st):
                # tanh + v-dot, then the online-softmax tile pass:
                #   et  = (e + 512) * m   (masked/pad -> 0; 512 > max|e| and
                #         exp(-512-max) underflows to exactly 0 in fp32,
                #         while ulp_f32(512)=6.1e-5 keeps e's precision)
                #   nmax_i = -max(et); ex = exp(et - max_i); z_i = sum(ex)
                #   part[:, dc, i] = sum_t ex_t * hT[p, dc, t]
                w = widths_of[b][tt]
                nmax, zs, _, _ = st
                e_ps = e_pool.tile([P, TT], F32, tag="e", name=f"e{b}_{tt}")
                ths = [None] * MC

                def emit_tanh(mc):
                    th = tanh_pool.tile(
                        [P, TT], F16, tag="th", name=f"th{b}_{tt}_{mc}"
                    )
                    nc.scalar.activation(
                        out=th[:, :w],
                        in_=pps[mc][:, :w],
                        func=AF.Tanh,
                        bias=proj_sb[:, mc, b : b + 1],
                        scale=1.0,
                    )
                    ths[mc] = th

                # DVE-side chunks FIRST: their tanh->chain path gates the
                # all-ones matmul, which heads the e_ps PSUM group
                # (start=True); the PE-side v-dots close the group.
                for mc in range(VDP, MC):
                    emit_tanh(mc)
                ea = vd_pool.tile([P, TT], F16, tag="ea", name=f"ea{b}_{tt}_a")
                nc.vector.tensor_scalar_mul(
                    out=ea[:, :w],
                    in0=ths[VDP][:, :w],
                    scalar1=v_col[:, VDP : VDP + 1],
                )
                for k in range(VDP + 1, MC):
                    ea2 = vd_pool.tile(
                        [P, TT], F16, tag="ea", name=f"ea{b}_{tt}_{k}"
                    )
                    nc.vector.scalar_tensor_tensor(
                        out=ea2[:, :w],
                        in0=ths[k][:, :w],
                        scalar=v_col[:, k : k + 1],
                        in1=ea[:, :w],
                        op0=ALU.mult,
                        op1=ALU.add,
                    )
                    ea = ea2
                nc.tensor.matmul(
                    e_ps[:, :w],
                    lhsT=ones_bc,
                    rhs=ea[:, :w],
                    start=True,
                    stop=False,
                )
                for mc in range(VDP):
                    emit_tanh(mc)
                    nc.tensor.matmul(
                        e_ps[:, :w],
                        lhsT=v_bc[:, mc, :],
                        rhs=ths[mc][:, :w],
                        start=False,
                        stop=(mc == VDP - 1),
                    )
                # et is F32: it holds e+512, and ulp_f16(512)=0.5 would
                # quantize the energy to +-0.25.
                et = p2_pool.tile([P, TT], F32, tag="et", name=f"et{b}_{tt}")
                nc.vector.scalar_tensor_tensor(
                    out=et[:, :w],
                    in0=e_ps[:, :w],
                    scalar=512.0,
                    in1=mb_sb[:, offs_of[b][tt] : offs_of[b][tt] + w],
                    op0=ALU.add,
                    op1=ALU.mult,
                )
                nc.vector.tensor_reduce(
                    out=nmax[:, tt : tt + 1],
                    in_=et[:, :w],
                    axis=mybir.AxisListType.X,
                    op=ALU.max,
                    negate=True,
                )
                ex = p2_pool.tile([P, TT], F16, tag="ex", name=f"ex{b}_{tt}")
                nc.scalar.activation(
                    out=ex[:, :w],
                    in_=et[:, :w],
                    func=AF.Exp,
                    bias=nmax[:, tt : tt + 1],
                    scale=1.0,
                    accum_out=zs[:, tt : tt + 1],
                )
                return ex

            def emit_tile_ctx(b, tt, ht_tiles, ex, st):
                # deferred one tile behind the softmax head so the next
                # tile's et/nmax (critical path to its exp) queue ahead of
                # these 8 accumulates on the DVE.
                w = widths_of[b][tt]
                _, _, part, scr = st
                for dc in range(KC):
                    nc.vector.scalar_tensor_tensor(
                        out=scr[:, :w],
                        in0=ht_tiles[tt][:, dc, :],
                        scalar=1.0,
                        in1=ex[:, :w],
                        op0=ALU.mult,
                        op1=ALU.mult,
                        accum_out=part[:, dc, tt : tt + 1],
                    )

            def emit_batch_tail(b, st):
                # combine tiles: f_i = exp(max_i - M) with global max M,
                # ctx = sum_i part_i f_i / sum_i z_i f_i  (all tiny tiles)
                nt = nt_of[b]
                nmax, zs, part, _ = st
                negM = small_pool.tile([P, 1], F32, tag="negM", name=f"nM{b}")
                nc.vector.tensor_reduce(
                    out=negM, in_=nmax, axis=mybir.AxisListType.X, op=ALU.min
                )
                f = small_pool.tile([P, nt], F32, tag=f"f{nt}", name=f"f{b}")
                nc.scalar.activation(
                    out=f, in_=nmax, func=AF.Exp, bias=negM, scale=-1.0
                )
                fz = small_pool.tile([P, nt], F32, tag=f"fz{nt}", name=f"fz{b}")
                zf = small_pool.tile([P, 1], F32, tag="zf", name=f"zf{b}")
                nc.vector.scalar_tensor_tensor(
                    out=fz,
                    in0=zs,
                    scalar=1.0,
                    in1=f,
                    op0=ALU.mult,
                    op1=ALU.mult,
                    accum_out=zf,
                )
                sinv = small_pool.tile([P, 1], F32, tag="sinv", name=f"si{b}")
                nc.vector.reciprocal(sinv, zf)
                for tt in range(nt):
                    nc.vector.tensor_scalar_mul(
                        out=part[:, :, tt : tt + 1],
                        in0=part[:, :, tt : tt + 1],
                        scalar1=f[:, tt : tt + 1],
                    )
                ctx = ctx_pool.tile([P, KC], F32, tag="ctx", name=f"cx{b}")
                nc.vector.tensor_reduce(
                    out=ctx, in_=part, axis=mybir.AxisListType.X, op=ALU.add
                )
                nc.vector.tensor_scalar_mul(out=ctx, in0=ctx, scalar1=sinv)
                nc.sync.dma_start(
                    out=out[b].rearrange("(p dc) -> p dc", p=P), in_=ctx
                )

            def batch_state(b):
                nt = nt_of[b]
                nmax = small_pool.tile([P, nt], F32, tag=f"nmax{nt}", name=f"nm{b}")
                zs = small_pool.tile([P, nt], F32, tag=f"zs{nt}", name=f"zs{b}")
                part = ctx_pool.tile(
                    [P, KC, nt], F32, tag=f"part{nt}", name=f"pt{b}"
                )
                scr = scr_pool.tile([P, TT], F16, tag="scr", name=f"sc{b}")
                return nmax, zs, part, scr

            # ---- pipeline -------------------------------------------------
            # batch b+1's DMAs are emitted BEFORE batch b's compute so the
            # transfers are in flight long before the PE reaches them.  Each
            # tile's ctx accumulates are emitted AFTER the next tile's
            # softmax head (deferred-ctx: keeps et/nmax ahead on the DVE).
            dmas = {0: emit_batch_dmas(0, pre_tiles=ht0_tiles)}
            pend = None  # (b, tt, ht_tiles, ex, st) awaiting ctx emission
            for b in range(BL):
                if b + 1 < BL:
                    dmas[b + 1] = emit_batch_dmas(b + 1)
                ht_tiles, mb_sb = dmas.pop(b)
                st = batch_state(b)
                for tt in range(nt_of[b]):
                    mc_order = MC0_ORDER if b == 0 and tt == 0 else None
                    pps = emit_mains(b, tt, ht_tiles, mc_order)
                    ex = emit_tile_soft(b, tt, pps, ht_tiles, mb_sb, st)
                    if pend is not None:
                        emit_tile_ctx(*pend)
                    pend = (b, tt, ht_tiles, ex, st)
                emit_tile_ctx(*pend)
                pend = None
                emit_batch_tail(b, st)

    nc.compile()
    return nc


_NC_CACHE = {}


def _get_module(widths):
    if widths not in _NC_CACHE:
        _NC_CACHE[widths] = build_module(widths)
    return _NC_CACHE[widths]


def core_in_map(s, h, mask, W, U, v, c, widths, assign):
    """Shard + pack unmasked timesteps + lay out the inputs for core c.

    hT is delivered pre-tiled: per T-tile contiguous (P, KC, w) tensors
    (partition-contiguous rows -> large DMA packets); U likewise as
    contiguous (P, KC, cols) column chunks.
    """
    plans = _plans(widths)
    im = {}
    proj_rows = np.empty((BL, H), np.float32)
    for bl in range(BL):
        gb = assign[c][bl]
        wd = widths[bl]
        h_b = np.asarray(h, np.float32)[gb]
        m_b = np.asarray(mask)[gb] != 0
        idx = np.nonzero(m_b)[0]
        tb = min(len(idx), wd)
        hT_p = np.zeros((D, wd), dtype=np.float16)
        mf_p = np.zeros((wd,), dtype=np.float16)
        if tb:
            hT_p[:, :tb] = h_b[idx[:tb], :].T.astype(np.float16)
            mf_p[:tb] = 1.0
        # (kc p) rows -> (P, KC, w) tiles
        hr = hT_p.reshape(KC, P, wd)
        ws = plans[bl]
        offs = [sum(ws[:i]) for i in range(len(ws))]
        for tt, w in enumerate(ws):
            im[f"hT{bl}_{tt}"] = np.ascontiguousarray(
                hr[:, :, offs[tt] : offs[tt] + w].transpose(1, 0, 2)
            )
        im[f"maskf{bl}"] = mf_p
        proj_rows[bl] = np.asarray(s, np.float32)[0, gb] @ np.asarray(
            W, np.float32
        )
    Ur = (
        np.asarray(U, np.float32)
        .astype(np.float16)
        .reshape(KC, P, H)
        .transpose(1, 0, 2)
    )
    for i, (lo, hi) in enumerate(UCHUNKS):
        im[f"U{i}"] = np.ascontiguousarray(Ur[:, :, lo:hi])
    # partition-major: proj_l[p, mc*BL + b] = proj[b, mc*128 + p]
    im["proj"] = np.ascontiguousarray(
        proj_rows.T.reshape(MC, P, BL).transpose(1, 0, 2).reshape(P, MC * BL)
    )
    # v_l[p, mc] = v[mc*128 + p]
    im["v"] = np.ascontiguousarray(
        np.asarray(v, np.float32).reshape(MC, P).T
    )
    return im


def unscramble_out(arr):
    """(BL, P*KC) device layout [p, dc] -> (BL, D) with d = dc*128 + p."""
    arr = np.asarray(arr)
    return np.ascontiguousarray(
        arr.reshape(-1, P, KC).transpose(0, 2, 1).reshape(-1, D)
    )


def assemble_out(results, mask, assign):
    """Scatter per-core slot outputs back to global batch order."""
    outp = np.zeros((B, D), np.float32)
    for c in range(NCORES):
        ob = unscramble_out(results[c]["out"])
        for bl in range(BL):
            outp[assign[c][bl]] = ob[bl]
    # fully-masked batches: reference yields exactly 0 (softmax uniform
    # over zeroed h); the device path divides by z=0 there, so overwrite.
    tb = np.asarray(mask).astype(bool).sum(axis=1)
    outp[tb == 0] = 0.0
    return outp


def kernel(s, h, mask, W, U, v):
    widths, assign = plan_from_mask(mask)
    in_maps = [
        core_in_map(s, h, mask, W, U, v, c, widths, assign)
        for c in range(NCORES)
    ]
    nc = _get_module(widths)
    res = run_bass_kernel_spmd(nc, in_maps, list(range(NCORES)))
    return assemble_out(res.results, mask, assign)
